# revision 12
# baseline (speedup 1.0000x reference)
"""Trainium2 Bass kernel for nn_BilinearBlock (bilinear attention + bilinear MLP).

Sharding: 8 cores = (batch b in 0..3) x (query-half h in 0..1), balanced causal
split via a host-side local sequence permutation so one uniform SPMD program
serves both halves (q blocks at local slots 0 and 2; 24 score pairs/core).

Precision (validated, ~7e-3 total rel err vs 2e-2 gate): first RMSNorm + score
scale pre-applied on host (xn bf16 + raw xq for the residual); attention in
bf16 (bf16 rope tables, f32 out1); MLP in fp8e4 DoubleRow (2x PE throughput)
with power-of-2 scales; second RMSNorm on device, folded into the fp8 scale.

Schedule: host-packed per-partition-contiguous inputs split across the three
DMA-issue queues (sync/scalar HWDGE + gpsimd SWDGE); PE warm-up matmuls at t=0
so HAM reaches 8/8 before real work; scores software-pipelined one pair ahead;
first MLP weight pairs prefetched during attention so the MN stage starts with
no DMA bubble; fp8 MLP runs dense at the DoubleRow roofline.
"""
import os
import sys

for _p in ('/opt/trn_rl_repo',):
    if _p not in sys.path:
        sys.path.insert(0, _p)

import numpy as np
import ml_dtypes

import concourse.bass as bass
import concourse.mybir as mybir
import concourse.tile as tile
from concourse import bacc
from concourse.bass_utils import run_bass_kernel_spmd
from concourse.masks import make_identity

P = 128
S = 2048
R = 1024          # query rows per core
D = 1024
DH = 128
DM = 4096
NT = 512
FC = D // P
KC = S // P
DMC = DM // P
NBLK = S // NT
EPS = 1e-6
SX = 16.0
SG = 4.0
SM8 = 16.0      # fp8 scale of the r^2-folded M-side operand
S28 = 0.25      # fp8 scale of the unnormalized N-side operand (|out1| peaks ~300)
F32 = mybir.dt.float32
F32R = mybir.dt.float32r
BF16 = mybir.dt.bfloat16
F8 = mybir.dt.float8e4
DR = mybir.MatmulPerfMode.DoubleRow
ALU = mybir.AluOpType
ACT = mybir.ActivationFunctionType

N_MASK = 16
PPB = 4       # pp-tag PSUM ring depth
NPRE = 2          # MN pairs run h0-only before h1 is ready (weights reloaded)
NPF = 3           # MN weight pairs prefetched during attention

LAST_EXEC_NS = None
_cached = {}


def _build(cpost, c_fin):
    nc = bacc.Bacc("TRN2", target_bir_lowering=False, debug=False, num_devices=8)

    xtb_d = nc.dram_tensor("xtb", [P, NBLK, FC, NT], BF16,
                           kind="ExternalInput").ap()
    xqp_d = nc.dram_tensor("xqp", [P, FC, R], BF16, kind="ExternalInput").ap()
    cs_d = nc.dram_tensor("cs", [P, NBLK, 2, NT], BF16,
                          kind="ExternalInput").ap()
    mask_d = nc.dram_tensor("mask_in", [P, N_MASK, NT], BF16,
                            kind="ExternalInput").ap()
    wkqv_d = nc.dram_tensor("wkqv", [P, 5, FC, DH], BF16,
                            kind="ExternalInput").ap()
    wo_d = nc.dram_tensor("wop", [DH, FC, P], BF16, kind="ExternalInput").ap()
    wm8_d = nc.dram_tensor("wm8p", [P, DMC // 2, FC, 2 * P], F8,
                           kind="ExternalInput").ap()
    wn8_d = nc.dram_tensor("wn8p", [P, DMC // 2, FC, 2 * P], F8,
                           kind="ExternalInput").ap()
    wp8_d = nc.dram_tensor("wp8p", [P, FC // 2, DMC, 2 * P], F8,
                           kind="ExternalInput").ap()
    outT = nc.dram_tensor("outT", [D, R], BF16, kind="ExternalOutput").ap()

    with tile.TileContext(nc) as tc:
        with tc.tile_pool(name="glob", bufs=1) as glob, \
             tc.tile_pool(name="keep", bufs=1) as keep, \
             tc.tile_pool(name="ktmp", bufs=2) as ktmp, \
             tc.tile_pool(name="dw", bufs=1) as dw, \
             tc.tile_pool(name="psacc", bufs=1, space="PSUM") as psacc:
            ident = glob.tile([P, P], BF16, tag="ident")
            make_identity(nc, ident)
            warm = glob.tile([P, P], BF16, tag="warm")
            nc.vector.memset(warm, 0.25)
            ones = glob.tile([P, 1], BF16, tag="ones")
            nc.vector.memset(ones, 1.0)
            epsD = glob.tile([1, 1], F32, tag="epsD")
            nc.vector.memset(epsD, EPS / (SX * SX))
            out1T = [glob.tile([P, R], F32, tag=f"o1_{f}", name=f"o1_{f}")
                     for f in range(FC)]
            gqs = glob.tile([P, R], F32, tag="gqs")
            rsb2 = glob.tile([1, R], F32, tag="rsb2")
            r2row = glob.tile([1, R], F32, tag="r2row")
            g_row = glob.tile([1, R], F32, tag="g_row")

            xq = keep.tile([P, FC, R], BF16, tag="xq")
            attnT = keep.tile([DH, R], BF16, tag="attnT")
            woblk = keep.tile([DH, FC, P], BF16, tag="wo")
            out18 = keep.tile([P, FC, R], F8, tag="out18")
            out18m = keep.tile([P, FC, R], F8, tag="out18m")

            # MN weight-pair load (ring bufs=NPF; prefetchable during attn)
            def load_mn(dp, eng):
                wm8t = dw.tile([P, FC, 2 * P], F8, tag="wm8", bufs=NPF)
                eng.dma_start(out=wm8t, in_=wm8_d[:, dp])
                wn8t = dw.tile([P, FC, 2 * P], F8, tag="wn8", bufs=NPF)
                eng.dma_start(out=wn8t, in_=wn8_d[:, dp])
                return wm8t, wn8t

            def c_half(hj, acc, psum_pool):
                """o_proj + residual + norm2 squares/sums for query half hj.

                Software-pipelined (lookahead 2): the acc matmul of step f
                waits on the DVE add + square chain, so pw matmuls of f+1/f+2
                are issued ahead of it to keep the PE stream dense.
                """
                hsl = slice(hj * NT, (hj + 1) * NT)

                def issue_pw(f):
                    pw = psum_pool.tile([P, NT], F32, tag="pp", bufs=PPB)
                    nc.tensor.matmul(pw, woblk[:, f], attnT[:, hsl],
                                     start=True, stop=True)
                    return pw

                def epi(f, pw):
                    nc.vector.tensor_add(out=out1T[f][:, hsl], in0=pw,
                                         in1=xq[:, f, hsl])
                    sq2 = ktmp.tile([P, NT], BF16, tag="sq2", bufs=3)
                    nc.gpsimd.tensor_mul(out=sq2, in0=out1T[f][:, hsl],
                                         in1=out1T[f][:, hsl])
                    nc.tensor.matmul(acc, ones, sq2,
                                     start=(f == 0), stop=(f == FC - 1))

                pws = [issue_pw(0), issue_pw(1)]
                for f in range(FC):
                    if f + 2 < FC:
                        pws.append(issue_pw(f + 2))
                    epi(f, pws[f])

            def chain(hj, acc):
                """sqrt -> recip -> partition broadcast for half hj."""
                jsl = slice(hj * NT, (hj + 1) * NT)
                nc.scalar.activation(out=rsb2[:, jsl], in_=acc,
                                     func=ACT.Sqrt, bias=epsD,
                                     scale=1.0 / (D * SX * SX))
                nc.vector.reciprocal_approx_fast(out=r2row[:, jsl],
                                                 in_=rsb2[:, jsl])
                # g = r^2 * SM8  (r = r2row / SX); folded into the M-side
                # fp8 operand so the MLP needs no per-column rescale.
                nc.scalar.activation(out=g_row[:, jsl], in_=r2row[:, jsl],
                                     func=ACT.Square, bias=0.0,
                                     scale=SM8 ** 0.5 / SX)
                nc.gpsimd.partition_broadcast(gqs[:, jsl], g_row[:, jsl],
                                              channels=P)

            # ================= attention scope =================
            with tc.tile_pool(name="asb", bufs=1) as asb, \
                 tc.tile_pool(name="atmp", bufs=2) as atmp:

                xt = asb.tile([P, NBLK, FC, NT], BF16, tag="xt")
                k1T = asb.tile([DH, S], BF16, tag="k1T")
                k2T = asb.tile([DH, S], BF16, tag="k2T")
                q1T = asb.tile([DH, R], BF16, tag="q1T")
                q2T = asb.tile([DH, R], BF16, tag="q2T")
                v_rm = [asb.tile([P, DH], BF16, tag=f"vrm{i}", name=f"vrm{i}")
                        for i in range(KC)]
                cs = asb.tile([P, NBLK, 2, NT], BF16, tag="cs")
                masks = asb.tile([P, N_MASK, NT], BF16, tag="masks")
                wkqv = asb.tile([P, 5, FC, DH], BF16, tag="wkqv")
                wblks = {"wk1": wkqv[:, 0], "wk2": wkqv[:, 1],
                         "wq1": wkqv[:, 2], "wq2": wkqv[:, 3],
                         "wv": wkqv[:, 4]}

                # ---- input DMAs.  Per-engine DMA cost is per ring
                # entry (~0.6us, 16KB max), so per-partition segments are
                # kept large and the critical stream owns the sync queue in
                # strict need order (in-queue order = completion order).
                nc.sync.dma_start(out=wkqv, in_=wkqv_d)
                nc.sync.dma_start(out=xt[:, 0], in_=xtb_d[:, 0])
                nc.sync.dma_start(out=xt[:, 1], in_=xtb_d[:, 1])
                nc.sync.dma_start(out=xt[:, 2:4], in_=xtb_d[:, 2:4])
                # scalar (HWDGE#2): non-critical loads off the xt queue.
                nc.scalar.dma_start(out=masks, in_=mask_d)
                nc.scalar.dma_start(out=xq, in_=xqp_d)
                nc.scalar.dma_start(out=woblk, in_=wo_d)
                # gpsimd (SWDGE): rope tables, off the critical queue.
                nc.gpsimd.dma_start(out=cs, in_=cs_d)
                mn_tiles = [load_mn(dp, nc.sync) for dp in range(NPF)]

                with tc.tile_pool(name="psA", bufs=1, space="PSUM") as psA:
                    # HAM warm-up: keep PE busy from t=0 so the clock gate
                    # opens (4/8 -> 8/8) before the first real matmul.
                    wrm = psA.tile([P, NT], F32, tag="av", bufs=1, name="wrm")
                    for _ in range(72):
                        nc.tensor.matmul(wrm[:, 0:P], warm, warm,
                                         start=True, stop=True)

                    def rope_proj(wname, blk, dstT, dst_sl, u_pool):
                        pp = psA.tile([P, NT], F32, tag="pp", bufs=PPB)
                        wb = wblks[wname]
                        for f in range(FC):
                            nc.tensor.matmul(pp, wb[:, f], xt[:, blk, f],
                                             start=(f == 0), stop=(f == FC - 1))
                        rot = atmp.tile([P, NT], F32, tag="rot", bufs=3)
                        nc.scalar.activation(out=rot[0:64], in_=pp[64:128],
                                             func=ACT.Copy, bias=0.0, scale=1.0)
                        nc.scalar.activation(out=rot[64:128], in_=pp[0:64],
                                             func=ACT.Copy, bias=0.0, scale=1.0)
                        t1 = atmp.tile([P, NT], F32, tag="t1", bufs=3)
                        nc.vector.tensor_mul(out=t1, in0=pp, in1=cs[:, blk, 0])
                        u = atmp.tile([P, NT], F32, tag="u", bufs=2)
                        if u_pool:
                            nc.gpsimd.tensor_mul(out=u, in0=rot,
                                                 in1=cs[:, blk, 1])
                        else:
                            nc.vector.tensor_mul(out=u, in0=rot,
                                                 in1=cs[:, blk, 1])
                        nc.gpsimd.tensor_add(out=dstT[:, dst_sl], in0=t1, in1=u)

                    def v_proj(blk):
                        pp = psA.tile([P, NT], F32, tag="pp", bufs=PPB)
                        wb = wblks["wv"]
                        for f in range(FC):
                            nc.tensor.matmul(pp, wb[:, f], xt[:, blk, f],
                                             start=(f == 0), stop=(f == FC - 1))
                        vt = atmp.tile([P, NT], BF16, tag="vt", bufs=2)
                        nc.scalar.activation(out=vt, in_=pp, func=ACT.Copy,
                                             bias=0.0, scale=1.0)
                        for t in range(NT // P):
                            tp = psA.tile([P, P], BF16, tag="tp", bufs=1)
                            nc.tensor.transpose(tp, vt[:, t * P:(t + 1) * P],
                                                ident)
                            nc.scalar.activation(out=v_rm[blk * 4 + t], in_=tp,
                                                 func=ACT.Copy, bias=0.0,
                                                 scale=1.0)

                    def scores(qb, npairs):
                        qsl = slice(qb * NT, (qb + 1) * NT)
                        avp = psA.tile([P, NT], F32, tag="av", bufs=1,
                                       name=f"av{qb}")

                        def issue_s(i):
                            ksl = slice(i * P, (i + 1) * P)
                            s1 = psA.tile([P, NT], F32, tag="pp", bufs=PPB,
                                          name=f"s1_{qb}_{i}")
                            nc.tensor.matmul(s1, k1T[:, ksl], q1T[:, qsl],
                                             start=True, stop=True)
                            s2 = psA.tile([P, NT], F32, tag="pp", bufs=PPB,
                                          name=f"s2_{qb}_{i}")
                            nc.tensor.matmul(s2, k2T[:, ksl], q2T[:, qsl],
                                             start=True, stop=True)
                            return s1, s2

                        def epilogue(i, s1, s2):
                            # TensorTensor ops may read only ONE input from
                            # PSUM: bounce s1 via the mask-mul (plane 16 is
                            # all-ones for unmasked pairs, keeping the scalar
                            # queue free for the copies on the critical path).
                            aT = atmp.tile([P, NT], BF16, tag="aT", bufs=4)
                            masked = (qb == 0) or (i >= 8)
                            sm = atmp.tile([P, NT], BF16, tag="sm", bufs=3)
                            if masked:
                                nc.vector.tensor_mul(
                                    out=sm, in0=s1,
                                    in1=masks[:, qb * 8 + (i % 8)])
                            else:
                                nc.scalar.activation(out=sm, in_=s1,
                                                     func=ACT.Copy, bias=0.0,
                                                     scale=1.0)
                            nc.vector.tensor_mul(out=aT, in0=sm, in1=s2)
                            nc.tensor.matmul(avp, v_rm[i], aT,
                                             start=(i == 0),
                                             stop=(i == npairs - 1))

                        prev = issue_s(0)
                        for i in range(npairs):
                            nxt = issue_s(i + 1) if i + 1 < npairs else None
                            epilogue(i, *prev)
                            prev = nxt
                        nc.scalar.activation(out=attnT[:, qsl], in_=avp,
                                             func=ACT.Copy, bias=0.0, scale=1.0)

                    def do_block(blk):
                        is_q = blk in (0, 2)
                        sl_blk = slice(blk * NT, (blk + 1) * NT)
                        rope_proj("wk1", blk, k1T, sl_blk, u_pool=True)
                        rope_proj("wk2", blk, k2T, sl_blk, u_pool=False)
                        if is_q:
                            qsl = slice((blk // 2) * NT, (blk // 2 + 1) * NT)
                            rope_proj("wq1", blk, q1T, qsl, u_pool=True)
                            rope_proj("wq2", blk, q2T, qsl, u_pool=False)
                        v_proj(blk)

                    do_block(0)
                    do_block(1)
                    do_block(2)
                    scores(0, 8)
                    do_block(3)
                    acc0 = psacc.tile([1, NT], F32, tag="acc", bufs=1,
                                      name="acc0")
                    c_half(0, acc0, psA)
                    chain(0, acc0)
                    for f in range(FC):          # fp8 operands, half 0
                        nc.scalar.activation(out=out18[:, f, 0:NT],
                                             in_=out1T[f][:, 0:NT],
                                             func=ACT.Copy, bias=0.0,
                                             scale=S28)
                        eng = nc.vector if f % 2 == 0 else nc.gpsimd
                        eng.tensor_mul(out=out18m[:, f, 0:NT],
                                       in0=out1T[f][:, 0:NT],
                                       in1=gqs[:, 0:NT])
                    scores(1, KC)
                    acc1 = psacc.tile([1, NT], F32, tag="acc", bufs=1,
                                      name="acc1")
                    c_half(1, acc1, psA)
                    chain(1, acc1)
                    for f in range(FC):          # fp8 operands, half 1
                        nc.scalar.activation(out=out18[:, f, NT:R],
                                             in_=out1T[f][:, NT:R],
                                             func=ACT.Copy, bias=0.0,
                                             scale=S28)
                        eng = nc.vector if f % 2 == 0 else nc.gpsimd
                        eng.tensor_mul(out=out18m[:, f, NT:R],
                                       in0=out1T[f][:, NT:R],
                                       in1=gqs[:, NT:R])

            # ================= phase D: fp8 MLP =================
            # MN order: (dp 0..NPRE-1, h0 only), (dp NPRE.., both), (dp
            # 0..NPRE-1, h1, reloaded) — the h0 prefix runs while the half-1
            # norm chain and xn8 finish on DVE/pool.
            with tc.tile_pool(name="dsb", bufs=1) as dsb, \
                 tc.tile_pool(name="dwp", bufs=1) as dwp, \
                 tc.tile_pool(name="dtmp", bufs=2) as dtmp:
                gts = dsb.tile([P, DMC, R], F8, tag="gts")

                sched = ([(dp, (0,)) for dp in range(NPRE)]
                         + [(dp, (0, 1)) for dp in range(NPRE, DMC // 2)]
                         + [(dp, (1,)) for dp in range(NPRE)])

                with tc.tile_pool(name="psD", bufs=1, space="PSUM") as psD:
                    def mn_pair(dp, hjs, tiles):
                        """two d_mlp chunks (one paired weight load) x halves."""
                        wm8t, wn8t = tiles
                        for s in range(2):
                            dmc = dp * 2 + s
                            msl = slice(s * P, (s + 1) * P)
                            for hj in hjs:
                                hsl = slice(hj * NT, (hj + 1) * NT)
                                mps = psD.tile([P, NT], F32, tag="mps", bufs=4)
                                for t in range(FC // 2):
                                    nc.tensor.matmul(
                                        mps, wm8t[:, 2 * t:2 * t + 2, msl],
                                        out18m[:, 2 * t:2 * t + 2, hsl],
                                        start=(t == 0), stop=(t == FC // 2 - 1),
                                        perf_mode=DR)
                                nps = psD.tile([P, NT], F32, tag="nps", bufs=3)
                                for t in range(FC // 2):
                                    nc.tensor.matmul(
                                        nps, wn8t[:, 2 * t:2 * t + 2, msl],
                                        out18[:, 2 * t:2 * t + 2, hsl],
                                        start=(t == 0), stop=(t == FC // 2 - 1),
                                        perf_mode=DR)
                                mpsc = dtmp.tile([P, NT], F32, tag="mcp",
                                                 bufs=3)
                                nc.scalar.activation(out=mpsc, in_=mps,
                                                     func=ACT.Copy, bias=0.0,
                                                     scale=cpost)
                                nc.vector.tensor_mul(out=gts[:, dmc, hsl],
                                                     in0=mpsc, in1=nps)

                    for j, (dp, hjs) in enumerate(sched):
                        mn_pair(dp, hjs, mn_tiles[j])
                        if j + NPF < len(sched):
                            mn_tiles.append(load_mn(sched[j + NPF][0], nc.sync))

                with tc.tile_pool(name="psWP", bufs=1, space="PSUM") as psWP:
                    for fp in range(FC // 2):
                        wp8t = dwp.tile([P, DMC, 2 * P], F8, tag="wp8", bufs=2)
                        nc.sync.dma_start(out=wp8t, in_=wp8_d[:, fp])
                        for s in range(2):
                            f = fp * 2 + s
                            fsl = slice(f * P, (f + 1) * P)
                            msl = slice(s * P, (s + 1) * P)
                            for hj in range(2):
                                hsl = slice(hj * NT, (hj + 1) * NT)
                                wps = psWP.tile([P, NT], F32, tag="wps", bufs=3)
                                for t in range(DMC // 2):
                                    nc.tensor.matmul(
                                        wps, wp8t[:, 2 * t:2 * t + 2, msl],
                                        gts[:, 2 * t:2 * t + 2, hsl],
                                        start=(t == 0),
                                        stop=(t == DMC // 2 - 1),
                                        perf_mode=DR)
                                fin = dtmp.tile([P, NT], BF16, tag="fin",
                                                bufs=3)
                                nc.vector.scalar_tensor_tensor(
                                    out=fin, in0=wps, scalar=c_fin,
                                    in1=out1T[f][:, hsl], op0=ALU.mult,
                                    op1=ALU.add)
                                nc.gpsimd.dma_start(out=outT[fsl, hsl], in_=fin)

    nc.compile()
    return nc


def _pow2_scale(w, target=120.0):
    m = float(np.abs(w).max())
    return 2.0 ** np.floor(np.log2(target / m))


def _f8(w, scale):
    return np.clip(np.asarray(w, np.float64) * scale, -240, 240).astype(
        ml_dtypes.float8_e4m3)


QB_COLS = [np.arange(NT), np.arange(NT) + 2 * NT]   # local q cols (blocks 0,2)


def _prepare(x, cos, sin, causal_mask, weights):
    """Host-side input prep. Returns in_maps + gather info + fp8 scales."""
    B = x.shape[0]
    coscat = np.concatenate([cos, cos], axis=1).T.astype(np.float32)   # [128,S]
    sincat = np.concatenate([-sin, sin], axis=1).T.astype(np.float32)
    valid = ~np.asarray(causal_mask, bool)          # valid[q, k] = k <= q

    wq1, wq2, wk1, wk2, wv, wo, wm, wn, wp = [np.asarray(w, np.float32)
                                              for w in weights]
    swm = _pow2_scale(wm)
    swn = _pow2_scale(wn)
    swp = _pow2_scale(wp)
    wm8 = _f8(wm, swm)
    wn8 = _f8(wn, swn)
    wp8 = _f8(wp, swp)
    cpost = SG / (swm * swn * SM8 * S28)
    c_fin = 1.0 / (SG * swp)

    bf = ml_dtypes.bfloat16
    scale = 1.0 / np.sqrt(DH)

    def pack_w(w):                                  # [D, DH] -> [P, FC, DH]
        return np.ascontiguousarray(
            w.reshape(FC, P, DH).transpose(1, 0, 2)).astype(bf)

    wkqv = np.ascontiguousarray(
        np.stack([pack_w(wk1), pack_w(wk2), pack_w(wq1 * scale),
                  pack_w(wq2 * scale), pack_w(wv)], axis=1))  # [P, 5, FC, DH]
    wop = np.ascontiguousarray(wo.reshape(DH, FC, P)).astype(bf)
    wm8p = np.ascontiguousarray(
        wm8.reshape(FC, P, DMC // 2, 2 * P).transpose(1, 2, 0, 3))
    wn8p = np.ascontiguousarray(
        wn8.reshape(FC, P, DMC // 2, 2 * P).transpose(1, 2, 0, 3))
    wp8p = np.ascontiguousarray(
        wp8.reshape(DMC, P, FC // 2, 2 * P).transpose(1, 2, 0, 3))

    # host-side first rmsnorm (exact f32)
    r_all = 1.0 / np.sqrt((x * x).mean(axis=-1, keepdims=True) + EPS)
    xn_all = x * r_all

    block_order = {0: [0, 1, 3, 2], 1: [1, 0, 2, 3]}
    in_maps = []
    qrows_per_core = []
    for c in range(8):
        b, h = c // 2, c % 2
        order = block_order[h]
        perm = np.concatenate([np.arange(NT) + NT * g for g in order])
        qrows = np.concatenate([perm[QB_COLS[0]], perm[QB_COLS[1]]])
        mask8 = np.zeros((P, N_MASK, NT), np.float32)
        for t in range(16):
            qb = 0 if t < 8 else 1
            qglob = perm[QB_COLS[qb]]
            kglob = perm[t * P:(t + 1) * P]
            # valid[q, k]; tile layout is [k, q]
            mask8[:, t, :] = valid[np.ix_(qglob, kglob)].T
        qrows_per_core.append((b, qrows))
        xnT = xn_all[b][perm].T                             # [D, S]
        xtb = np.ascontiguousarray(
            xnT.reshape(FC, P, NBLK, NT).transpose(1, 2, 0, 3)).astype(bf)
        xqT = x[b][qrows].T                                 # [D, R]
        xqp = np.ascontiguousarray(
            xqT.reshape(FC, P, R).transpose(1, 0, 2)).astype(bf)
        cosp = coscat[:, perm]
        sinp = sincat[:, perm]
        cs = np.ascontiguousarray(
            np.stack([cosp.reshape(P, NBLK, NT), sinp.reshape(P, NBLK, NT)],
                     axis=2)).astype(bf)                    # [P, NBLK, 2, NT]
        in_maps.append({
            "xtb": xtb, "xqp": xqp, "cs": cs,
            "mask_in": mask8.astype(bf),
            "wkqv": wkqv, "wop": wop,
            "wm8p": wm8p, "wn8p": wn8p, "wp8p": wp8p,
        })
    return in_maps, qrows_per_core, cpost, c_fin


def kernel(x, cos, sin, causal_mask, wq1, wq2, wk1, wk2, wv, wo, wm, wn, wp):
    global LAST_EXEC_NS
    x = np.asarray(x, dtype=np.float32)
    cos = np.asarray(cos, dtype=np.float32)
    sin = np.asarray(sin, dtype=np.float32)
    B = x.shape[0]

    in_maps, qrows_per_core, cpost, c_fin = _prepare(
        x, cos, sin, causal_mask,
        (wq1, wq2, wk1, wk2, wv, wo, wm, wn, wp))

    key = ("nc", float(cpost), float(c_fin))
    if key not in _cached:
        _cached.clear()
        _cached[key] = _build(float(cpost), float(c_fin))
    nc = _cached[key]

    trace = bool(os.environ.get("BASSK_TRACE"))
    if trace:
        _install_trace_hook()
    res = run_bass_kernel_spmd(nc, in_maps, core_ids=list(range(8)),
                               trace=trace)
    LAST_EXEC_NS = res.exec_time_ns

    out = np.empty((B, S, D), dtype=np.float32)
    for c in range(8):
        b, qrows = qrows_per_core[c]
        out[b, qrows, :] = np.asarray(res.results[c]["outT"]).T.astype(
            np.float32)
    return out


def _install_trace_hook():
    import types
    import antenv
    if getattr(antenv, "axon_hooks", None) is not None:
        return
    holder = {}
    m = types.ModuleType("antenv.axon_hooks")
    m.set_axon_ntff_profile_hook = lambda h: holder.__setitem__('h', h)
    m.get_axon_ntff_profile_hook = lambda: holder.get('h')
    sys.modules["antenv.axon_hooks"] = m
    antenv.axon_hooks = m
    from trn_agent_boot.trn_boot import _ntff_profile_via_ctypes
    m.set_axon_ntff_profile_hook(_ntff_profile_via_ctypes('/opt/axon/libaxon_pjrt.so'))


# revision 13
# speedup vs baseline: 1.0062x; 1.0062x over previous
"""Trainium2 Bass kernel for nn_BilinearBlock (bilinear attention + bilinear MLP).

Sharding: 8 cores = (batch b in 0..3) x (query-half h in 0..1), balanced causal
split via a host-side local sequence permutation so one uniform SPMD program
serves both halves (q blocks at local slots 0 and 2; 24 score pairs/core).

Precision (validated, ~7e-3 total rel err vs 2e-2 gate): first RMSNorm + score
scale pre-applied on host (xn bf16 + raw xq for the residual); attention in
bf16 (bf16 rope tables, f32 out1); MLP in fp8e4 DoubleRow (2x PE throughput)
with power-of-2 scales; second RMSNorm on device, folded into the fp8 scale.

Schedule: host-packed per-partition-contiguous inputs split across the three
DMA-issue queues (sync/scalar HWDGE + gpsimd SWDGE); PE warm-up matmuls at t=0
so HAM reaches 8/8 before real work; scores software-pipelined one pair ahead;
first MLP weight pairs prefetched during attention so the MN stage starts with
no DMA bubble; fp8 MLP runs dense at the DoubleRow roofline.
"""
import os
import sys

for _p in ('/opt/trn_rl_repo',):
    if _p not in sys.path:
        sys.path.insert(0, _p)

import numpy as np
import ml_dtypes

import concourse.bass as bass
import concourse.mybir as mybir
import concourse.tile as tile
from concourse import bacc
from concourse.bass_utils import run_bass_kernel_spmd
from concourse.masks import make_identity

P = 128
S = 2048
R = 1024          # query rows per core
D = 1024
DH = 128
DM = 4096
NT = 512
FC = D // P
KC = S // P
DMC = DM // P
NBLK = S // NT
EPS = 1e-6
SX = 16.0
SG = 4.0
SM8 = 16.0      # fp8 scale of the r^2-folded M-side operand
S28 = 0.25      # fp8 scale of the unnormalized N-side operand (|out1| peaks ~300)
F32 = mybir.dt.float32
F32R = mybir.dt.float32r
BF16 = mybir.dt.bfloat16
F8 = mybir.dt.float8e4
DR = mybir.MatmulPerfMode.DoubleRow
ALU = mybir.AluOpType
ACT = mybir.ActivationFunctionType

N_MASK = 16
PPB = 4       # pp-tag PSUM ring depth
NPRE = 2          # MN pairs run h0-only before h1 is ready (weights reloaded)
NPF = 3           # MN weight pairs prefetched during attention

LAST_EXEC_NS = None
_cached = {}


def _build(cpost, c_fin):
    nc = bacc.Bacc("TRN2", target_bir_lowering=False, debug=False, num_devices=8)

    xtb_d = nc.dram_tensor("xtb", [P, NBLK, FC, NT], BF16,
                           kind="ExternalInput").ap()
    xqp_d = nc.dram_tensor("xqp", [P, FC, R], BF16, kind="ExternalInput").ap()
    cs_d = nc.dram_tensor("cs", [P, NBLK, 2, NT], BF16,
                          kind="ExternalInput").ap()
    mask_d = nc.dram_tensor("mask_in", [P, N_MASK, NT], BF16,
                            kind="ExternalInput").ap()
    wkqv_d = nc.dram_tensor("wkqv", [P, 5, FC, DH], BF16,
                            kind="ExternalInput").ap()
    wo_d = nc.dram_tensor("wop", [DH, FC, P], BF16, kind="ExternalInput").ap()
    wm8_d = nc.dram_tensor("wm8p", [P, DMC // 2, FC, 2 * P], F8,
                           kind="ExternalInput").ap()
    wn8_d = nc.dram_tensor("wn8p", [P, DMC // 2, FC, 2 * P], F8,
                           kind="ExternalInput").ap()
    wp8_d = nc.dram_tensor("wp8p", [P, FC // 2, DMC, 2 * P], F8,
                           kind="ExternalInput").ap()
    outT = nc.dram_tensor("outT", [D, R], BF16, kind="ExternalOutput").ap()

    with tile.TileContext(nc) as tc:
        with tc.tile_pool(name="glob", bufs=1) as glob, \
             tc.tile_pool(name="keep", bufs=1) as keep, \
             tc.tile_pool(name="ktmp", bufs=2) as ktmp, \
             tc.tile_pool(name="dw", bufs=1) as dw, \
             tc.tile_pool(name="psacc", bufs=1, space="PSUM") as psacc:
            ident = glob.tile([P, P], BF16, tag="ident")
            make_identity(nc, ident)
            warm = glob.tile([P, P], BF16, tag="warm")
            nc.vector.memset(warm, 0.25)
            ones = glob.tile([P, 1], BF16, tag="ones")
            nc.vector.memset(ones, 1.0)
            epsD = glob.tile([1, 1], F32, tag="epsD")
            nc.vector.memset(epsD, EPS / (SX * SX))
            out1T = [glob.tile([P, R], F32, tag=f"o1_{f}", name=f"o1_{f}")
                     for f in range(FC)]
            gqs = glob.tile([P, R], F32, tag="gqs")
            rsb2 = glob.tile([1, R], F32, tag="rsb2")
            r2row = glob.tile([1, R], F32, tag="r2row")
            g_row = glob.tile([1, R], F32, tag="g_row")

            xq = keep.tile([P, FC, R], BF16, tag="xq")
            attnT = keep.tile([DH, R], BF16, tag="attnT")
            woblk = keep.tile([DH, FC, P], BF16, tag="wo")
            out18 = keep.tile([P, FC, R], F8, tag="out18")
            out18m = keep.tile([P, FC, R], F8, tag="out18m")

            # MN weight-pair load (ring bufs=NPF; prefetchable during attn)
            def load_mn(dp, eng):
                wm8t = dw.tile([P, FC, 2 * P], F8, tag="wm8", bufs=NPF)
                eng.dma_start(out=wm8t, in_=wm8_d[:, dp])
                wn8t = dw.tile([P, FC, 2 * P], F8, tag="wn8", bufs=NPF)
                eng.dma_start(out=wn8t, in_=wn8_d[:, dp])
                return wm8t, wn8t

            def c_half(hj, acc, psum_pool):
                """o_proj + residual + norm2 squares/sums for query half hj.

                Software-pipelined (lookahead 2): the acc matmul of step f
                waits on the DVE add + square chain, so pw matmuls of f+1/f+2
                are issued ahead of it to keep the PE stream dense.
                """
                hsl = slice(hj * NT, (hj + 1) * NT)

                def issue_pw(f):
                    pw = psum_pool.tile([P, NT], F32, tag="pp", bufs=PPB)
                    nc.tensor.matmul(pw, woblk[:, f], attnT[:, hsl],
                                     start=True, stop=True)
                    return pw

                def epi(f, pw):
                    nc.vector.tensor_add(out=out1T[f][:, hsl], in0=pw,
                                         in1=xq[:, f, hsl])
                    sq2 = ktmp.tile([P, NT], BF16, tag="sq2", bufs=3)
                    if f % 2 == 0:
                        nc.scalar.activation(out=sq2, in_=out1T[f][:, hsl],
                                             func=ACT.Square, bias=0.0,
                                             scale=1.0)
                    else:
                        nc.gpsimd.tensor_mul(out=sq2, in0=out1T[f][:, hsl],
                                             in1=out1T[f][:, hsl])
                    nc.tensor.matmul(acc, ones, sq2,
                                     start=(f == 0), stop=(f == FC - 1))

                pws = [issue_pw(0), issue_pw(1)]
                for f in range(FC):
                    if f + 2 < FC:
                        pws.append(issue_pw(f + 2))
                    epi(f, pws[f])

            def chain(hj, acc):
                """sqrt -> recip -> partition broadcast for half hj."""
                jsl = slice(hj * NT, (hj + 1) * NT)
                nc.scalar.activation(out=rsb2[:, jsl], in_=acc,
                                     func=ACT.Sqrt, bias=epsD,
                                     scale=1.0 / (D * SX * SX))
                nc.vector.reciprocal_approx_fast(out=r2row[:, jsl],
                                                 in_=rsb2[:, jsl])
                # g = r^2 * SM8  (r = r2row / SX); folded into the M-side
                # fp8 operand so the MLP needs no per-column rescale.
                nc.scalar.activation(out=g_row[:, jsl], in_=r2row[:, jsl],
                                     func=ACT.Square, bias=0.0,
                                     scale=SM8 ** 0.5 / SX)
                nc.gpsimd.partition_broadcast(gqs[:, jsl], g_row[:, jsl],
                                              channels=P)

            # ================= attention scope =================
            with tc.tile_pool(name="asb", bufs=1) as asb, \
                 tc.tile_pool(name="atmp", bufs=2) as atmp:

                xt = asb.tile([P, NBLK, FC, NT], BF16, tag="xt")
                k1T = asb.tile([DH, S], BF16, tag="k1T")
                k2T = asb.tile([DH, S], BF16, tag="k2T")
                q1T = asb.tile([DH, R], BF16, tag="q1T")
                q2T = asb.tile([DH, R], BF16, tag="q2T")
                v_rm = [asb.tile([P, DH], BF16, tag=f"vrm{i}", name=f"vrm{i}")
                        for i in range(KC)]
                cs = asb.tile([P, NBLK, 2, NT], BF16, tag="cs")
                masks = asb.tile([P, N_MASK, NT], BF16, tag="masks")
                wkqv = asb.tile([P, 5, FC, DH], BF16, tag="wkqv")
                wblks = {"wk1": wkqv[:, 0], "wk2": wkqv[:, 1],
                         "wq1": wkqv[:, 2], "wq2": wkqv[:, 3],
                         "wv": wkqv[:, 4]}

                # ---- input DMAs.  Per-engine DMA cost is per ring
                # entry (~0.6us, 16KB max), so per-partition segments are
                # kept large and the critical stream owns the sync queue in
                # strict need order (in-queue order = completion order).
                # gpsimd (SWDGE, best per-packet rate): the critical
                # first blocks + rope tables.
                nc.gpsimd.dma_start(out=xt[:, 0], in_=xtb_d[:, 0])
                nc.gpsimd.dma_start(out=cs, in_=cs_d)
                nc.gpsimd.dma_start(out=xt[:, 1], in_=xtb_d[:, 1])
                # sync (HWDGE): weights first, then the rest in need order.
                nc.sync.dma_start(out=wkqv, in_=wkqv_d)
                nc.sync.dma_start(out=xt[:, 2:4], in_=xtb_d[:, 2:4])
                nc.sync.dma_start(out=masks, in_=mask_d)
                nc.sync.dma_start(out=xq, in_=xqp_d)
                nc.sync.dma_start(out=woblk, in_=wo_d)
                mn_tiles = [load_mn(dp, nc.sync) for dp in range(NPF)]

                with tc.tile_pool(name="psA", bufs=1, space="PSUM") as psA:
                    # HAM warm-up: keep PE busy from t=0 so the clock gate
                    # opens (4/8 -> 8/8) before the first real matmul.
                    wrm = psA.tile([P, NT], F32, tag="av", bufs=1, name="wrm")
                    for _ in range(40):
                        nc.tensor.matmul(wrm[:, 0:P], warm, warm,
                                         start=True, stop=True)

                    def rope_proj(wname, blk, dstT, dst_sl, u_pool):
                        pp = psA.tile([P, NT], F32, tag="pp", bufs=PPB)
                        wb = wblks[wname]
                        for f in range(FC):
                            nc.tensor.matmul(pp, wb[:, f], xt[:, blk, f],
                                             start=(f == 0), stop=(f == FC - 1))
                        rot = atmp.tile([P, NT], F32, tag="rot", bufs=3)
                        nc.scalar.activation(out=rot[0:64], in_=pp[64:128],
                                             func=ACT.Copy, bias=0.0, scale=1.0)
                        nc.scalar.activation(out=rot[64:128], in_=pp[0:64],
                                             func=ACT.Copy, bias=0.0, scale=1.0)
                        t1 = atmp.tile([P, NT], F32, tag="t1", bufs=3)
                        nc.vector.tensor_mul(out=t1, in0=pp, in1=cs[:, blk, 0])
                        u = atmp.tile([P, NT], F32, tag="u", bufs=2)
                        if u_pool:
                            nc.gpsimd.tensor_mul(out=u, in0=rot,
                                                 in1=cs[:, blk, 1])
                        else:
                            nc.vector.tensor_mul(out=u, in0=rot,
                                                 in1=cs[:, blk, 1])
                        nc.gpsimd.tensor_add(out=dstT[:, dst_sl], in0=t1, in1=u)

                    def v_proj(blk):
                        pp = psA.tile([P, NT], F32, tag="pp", bufs=PPB)
                        wb = wblks["wv"]
                        for f in range(FC):
                            nc.tensor.matmul(pp, wb[:, f], xt[:, blk, f],
                                             start=(f == 0), stop=(f == FC - 1))
                        vt = atmp.tile([P, NT], BF16, tag="vt", bufs=2)
                        nc.scalar.activation(out=vt, in_=pp, func=ACT.Copy,
                                             bias=0.0, scale=1.0)
                        for t in range(NT // P):
                            tp = psA.tile([P, P], BF16, tag="tp", bufs=1)
                            nc.tensor.transpose(tp, vt[:, t * P:(t + 1) * P],
                                                ident)
                            nc.scalar.activation(out=v_rm[blk * 4 + t], in_=tp,
                                                 func=ACT.Copy, bias=0.0,
                                                 scale=1.0)

                    def scores(qb, npairs):
                        qsl = slice(qb * NT, (qb + 1) * NT)
                        avp = psA.tile([P, NT], F32, tag="av", bufs=1,
                                       name=f"av{qb}")

                        def issue_s(i):
                            ksl = slice(i * P, (i + 1) * P)
                            s1 = psA.tile([P, NT], F32, tag="pp", bufs=PPB,
                                          name=f"s1_{qb}_{i}")
                            nc.tensor.matmul(s1, k1T[:, ksl], q1T[:, qsl],
                                             start=True, stop=True)
                            s2 = psA.tile([P, NT], F32, tag="pp", bufs=PPB,
                                          name=f"s2_{qb}_{i}")
                            nc.tensor.matmul(s2, k2T[:, ksl], q2T[:, qsl],
                                             start=True, stop=True)
                            return s1, s2

                        def epilogue(i, s1, s2):
                            # TensorTensor ops may read only ONE input from
                            # PSUM: bounce s1 via the mask-mul (plane 16 is
                            # all-ones for unmasked pairs, keeping the scalar
                            # queue free for the copies on the critical path).
                            aT = atmp.tile([P, NT], BF16, tag="aT", bufs=4)
                            masked = (qb == 0) or (i >= 8)
                            sm = atmp.tile([P, NT], BF16, tag="sm", bufs=3)
                            if masked:
                                nc.vector.tensor_mul(
                                    out=sm, in0=s1,
                                    in1=masks[:, qb * 8 + (i % 8)])
                            else:
                                nc.scalar.activation(out=sm, in_=s1,
                                                     func=ACT.Copy, bias=0.0,
                                                     scale=1.0)
                            nc.vector.tensor_mul(out=aT, in0=sm, in1=s2)
                            nc.tensor.matmul(avp, v_rm[i], aT,
                                             start=(i == 0),
                                             stop=(i == npairs - 1))

                        prev = issue_s(0)
                        for i in range(npairs):
                            nxt = issue_s(i + 1) if i + 1 < npairs else None
                            epilogue(i, *prev)
                            prev = nxt
                        nc.scalar.activation(out=attnT[:, qsl], in_=avp,
                                             func=ACT.Copy, bias=0.0, scale=1.0)

                    def do_block(blk):
                        is_q = blk in (0, 2)
                        sl_blk = slice(blk * NT, (blk + 1) * NT)
                        rope_proj("wk1", blk, k1T, sl_blk, u_pool=True)
                        rope_proj("wk2", blk, k2T, sl_blk, u_pool=False)
                        if is_q:
                            qsl = slice((blk // 2) * NT, (blk // 2 + 1) * NT)
                            rope_proj("wq1", blk, q1T, qsl, u_pool=True)
                            rope_proj("wq2", blk, q2T, qsl, u_pool=False)
                        v_proj(blk)

                    do_block(0)
                    do_block(1)
                    do_block(2)
                    scores(0, 8)
                    do_block(3)
                    acc0 = psacc.tile([1, NT], F32, tag="acc", bufs=1,
                                      name="acc0")
                    c_half(0, acc0, psA)
                    chain(0, acc0)
                    for f in range(FC):          # fp8 operands, half 0
                        nc.scalar.activation(out=out18[:, f, 0:NT],
                                             in_=out1T[f][:, 0:NT],
                                             func=ACT.Copy, bias=0.0,
                                             scale=S28)
                        eng = nc.vector if f % 2 == 0 else nc.gpsimd
                        eng.tensor_mul(out=out18m[:, f, 0:NT],
                                       in0=out1T[f][:, 0:NT],
                                       in1=gqs[:, 0:NT])
                    scores(1, KC)
                    acc1 = psacc.tile([1, NT], F32, tag="acc", bufs=1,
                                      name="acc1")
                    c_half(1, acc1, psA)
                    chain(1, acc1)
                    for f in range(FC):          # fp8 operands, half 1
                        nc.scalar.activation(out=out18[:, f, NT:R],
                                             in_=out1T[f][:, NT:R],
                                             func=ACT.Copy, bias=0.0,
                                             scale=S28)
                        eng = nc.vector if f % 2 == 0 else nc.gpsimd
                        eng.tensor_mul(out=out18m[:, f, NT:R],
                                       in0=out1T[f][:, NT:R],
                                       in1=gqs[:, NT:R])

            # ================= phase D: fp8 MLP =================
            # MN order: (dp 0..NPRE-1, h0 only), (dp NPRE.., both), (dp
            # 0..NPRE-1, h1, reloaded) — the h0 prefix runs while the half-1
            # norm chain and xn8 finish on DVE/pool.
            with tc.tile_pool(name="dsb", bufs=1) as dsb, \
                 tc.tile_pool(name="dwp", bufs=1) as dwp, \
                 tc.tile_pool(name="dtmp", bufs=2) as dtmp:
                gts = dsb.tile([P, DMC, R], F8, tag="gts")

                sched = ([(dp, (0,)) for dp in range(NPRE)]
                         + [(dp, (0, 1)) for dp in range(NPRE, DMC // 2)]
                         + [(dp, (1,)) for dp in range(NPRE)])

                with tc.tile_pool(name="psD", bufs=1, space="PSUM") as psD:
                    def mn_pair(dp, hjs, tiles):
                        """two d_mlp chunks (one paired weight load) x halves."""
                        wm8t, wn8t = tiles
                        for s in range(2):
                            dmc = dp * 2 + s
                            msl = slice(s * P, (s + 1) * P)
                            for hj in hjs:
                                hsl = slice(hj * NT, (hj + 1) * NT)
                                mps = psD.tile([P, NT], F32, tag="mps", bufs=4)
                                for t in range(FC // 2):
                                    nc.tensor.matmul(
                                        mps, wm8t[:, 2 * t:2 * t + 2, msl],
                                        out18m[:, 2 * t:2 * t + 2, hsl],
                                        start=(t == 0), stop=(t == FC // 2 - 1),
                                        perf_mode=DR)
                                nps = psD.tile([P, NT], F32, tag="nps", bufs=3)
                                for t in range(FC // 2):
                                    nc.tensor.matmul(
                                        nps, wn8t[:, 2 * t:2 * t + 2, msl],
                                        out18[:, 2 * t:2 * t + 2, hsl],
                                        start=(t == 0), stop=(t == FC // 2 - 1),
                                        perf_mode=DR)
                                mpsc = dtmp.tile([P, NT], F32, tag="mcp",
                                                 bufs=3)
                                nc.scalar.activation(out=mpsc, in_=mps,
                                                     func=ACT.Copy, bias=0.0,
                                                     scale=cpost)
                                nc.vector.tensor_mul(out=gts[:, dmc, hsl],
                                                     in0=mpsc, in1=nps)

                    for j, (dp, hjs) in enumerate(sched):
                        mn_pair(dp, hjs, mn_tiles[j])
                        if j + NPF < len(sched):
                            mn_tiles.append(load_mn(sched[j + NPF][0], nc.sync))

                with tc.tile_pool(name="psWP", bufs=1, space="PSUM") as psWP:
                    for fp in range(FC // 2):
                        wp8t = dwp.tile([P, DMC, 2 * P], F8, tag="wp8", bufs=2)
                        nc.sync.dma_start(out=wp8t, in_=wp8_d[:, fp])
                        for s in range(2):
                            f = fp * 2 + s
                            fsl = slice(f * P, (f + 1) * P)
                            msl = slice(s * P, (s + 1) * P)
                            for hj in range(2):
                                hsl = slice(hj * NT, (hj + 1) * NT)
                                wps = psWP.tile([P, NT], F32, tag="wps", bufs=3)
                                for t in range(DMC // 2):
                                    nc.tensor.matmul(
                                        wps, wp8t[:, 2 * t:2 * t + 2, msl],
                                        gts[:, 2 * t:2 * t + 2, hsl],
                                        start=(t == 0),
                                        stop=(t == DMC // 2 - 1),
                                        perf_mode=DR)
                                fin = dtmp.tile([P, NT], BF16, tag="fin",
                                                bufs=3)
                                nc.vector.scalar_tensor_tensor(
                                    out=fin, in0=wps, scalar=c_fin,
                                    in1=out1T[f][:, hsl], op0=ALU.mult,
                                    op1=ALU.add)
                                nc.gpsimd.dma_start(out=outT[fsl, hsl], in_=fin)

    nc.compile()
    return nc


def _pow2_scale(w, target=120.0):
    m = float(np.abs(w).max())
    return 2.0 ** np.floor(np.log2(target / m))


def _f8(w, scale):
    return np.clip(np.asarray(w, np.float64) * scale, -240, 240).astype(
        ml_dtypes.float8_e4m3)


QB_COLS = [np.arange(NT), np.arange(NT) + 2 * NT]   # local q cols (blocks 0,2)


def _prepare(x, cos, sin, causal_mask, weights):
    """Host-side input prep. Returns in_maps + gather info + fp8 scales."""
    B = x.shape[0]
    coscat = np.concatenate([cos, cos], axis=1).T.astype(np.float32)   # [128,S]
    sincat = np.concatenate([-sin, sin], axis=1).T.astype(np.float32)
    valid = ~np.asarray(causal_mask, bool)          # valid[q, k] = k <= q

    wq1, wq2, wk1, wk2, wv, wo, wm, wn, wp = [np.asarray(w, np.float32)
                                              for w in weights]
    swm = _pow2_scale(wm)
    swn = _pow2_scale(wn)
    swp = _pow2_scale(wp)
    wm8 = _f8(wm, swm)
    wn8 = _f8(wn, swn)
    wp8 = _f8(wp, swp)
    cpost = SG / (swm * swn * SM8 * S28)
    c_fin = 1.0 / (SG * swp)

    bf = ml_dtypes.bfloat16
    scale = 1.0 / np.sqrt(DH)

    def pack_w(w):                                  # [D, DH] -> [P, FC, DH]
        return np.ascontiguousarray(
            w.reshape(FC, P, DH).transpose(1, 0, 2)).astype(bf)

    wkqv = np.ascontiguousarray(
        np.stack([pack_w(wk1), pack_w(wk2), pack_w(wq1 * scale),
                  pack_w(wq2 * scale), pack_w(wv)], axis=1))  # [P, 5, FC, DH]
    wop = np.ascontiguousarray(wo.reshape(DH, FC, P)).astype(bf)
    wm8p = np.ascontiguousarray(
        wm8.reshape(FC, P, DMC // 2, 2 * P).transpose(1, 2, 0, 3))
    wn8p = np.ascontiguousarray(
        wn8.reshape(FC, P, DMC // 2, 2 * P).transpose(1, 2, 0, 3))
    wp8p = np.ascontiguousarray(
        wp8.reshape(DMC, P, FC // 2, 2 * P).transpose(1, 2, 0, 3))

    # host-side first rmsnorm (exact f32)
    r_all = 1.0 / np.sqrt((x * x).mean(axis=-1, keepdims=True) + EPS)
    xn_all = x * r_all

    block_order = {0: [0, 1, 3, 2], 1: [1, 0, 2, 3]}
    in_maps = []
    qrows_per_core = []
    for c in range(8):
        b, h = c // 2, c % 2
        order = block_order[h]
        perm = np.concatenate([np.arange(NT) + NT * g for g in order])
        qrows = np.concatenate([perm[QB_COLS[0]], perm[QB_COLS[1]]])
        mask8 = np.zeros((P, N_MASK, NT), np.float32)
        for t in range(16):
            qb = 0 if t < 8 else 1
            qglob = perm[QB_COLS[qb]]
            kglob = perm[t * P:(t + 1) * P]
            # valid[q, k]; tile layout is [k, q]
            mask8[:, t, :] = valid[np.ix_(qglob, kglob)].T
        qrows_per_core.append((b, qrows))
        xnT = xn_all[b][perm].T                             # [D, S]
        xtb = np.ascontiguousarray(
            xnT.reshape(FC, P, NBLK, NT).transpose(1, 2, 0, 3)).astype(bf)
        xqT = x[b][qrows].T                                 # [D, R]
        xqp = np.ascontiguousarray(
            xqT.reshape(FC, P, R).transpose(1, 0, 2)).astype(bf)
        cosp = coscat[:, perm]
        sinp = sincat[:, perm]
        cs = np.ascontiguousarray(
            np.stack([cosp.reshape(P, NBLK, NT), sinp.reshape(P, NBLK, NT)],
                     axis=2)).astype(bf)                    # [P, NBLK, 2, NT]
        in_maps.append({
            "xtb": xtb, "xqp": xqp, "cs": cs,
            "mask_in": mask8.astype(bf),
            "wkqv": wkqv, "wop": wop,
            "wm8p": wm8p, "wn8p": wn8p, "wp8p": wp8p,
        })
    return in_maps, qrows_per_core, cpost, c_fin


def kernel(x, cos, sin, causal_mask, wq1, wq2, wk1, wk2, wv, wo, wm, wn, wp):
    global LAST_EXEC_NS
    x = np.asarray(x, dtype=np.float32)
    cos = np.asarray(cos, dtype=np.float32)
    sin = np.asarray(sin, dtype=np.float32)
    B = x.shape[0]

    in_maps, qrows_per_core, cpost, c_fin = _prepare(
        x, cos, sin, causal_mask,
        (wq1, wq2, wk1, wk2, wv, wo, wm, wn, wp))

    key = ("nc", float(cpost), float(c_fin))
    if key not in _cached:
        _cached.clear()
        _cached[key] = _build(float(cpost), float(c_fin))
    nc = _cached[key]

    trace = bool(os.environ.get("BASSK_TRACE"))
    if trace:
        _install_trace_hook()
    res = run_bass_kernel_spmd(nc, in_maps, core_ids=list(range(8)),
                               trace=trace)
    LAST_EXEC_NS = res.exec_time_ns

    out = np.empty((B, S, D), dtype=np.float32)
    for c in range(8):
        b, qrows = qrows_per_core[c]
        out[b, qrows, :] = np.asarray(res.results[c]["outT"]).T.astype(
            np.float32)
    return out


def _install_trace_hook():
    import types
    import antenv
    if getattr(antenv, "axon_hooks", None) is not None:
        return
    holder = {}
    m = types.ModuleType("antenv.axon_hooks")
    m.set_axon_ntff_profile_hook = lambda h: holder.__setitem__('h', h)
    m.get_axon_ntff_profile_hook = lambda: holder.get('h')
    sys.modules["antenv.axon_hooks"] = m
    antenv.axon_hooks = m
    from trn_agent_boot.trn_boot import _ntff_profile_via_ctypes
    m.set_axon_ntff_profile_hook(_ntff_profile_via_ctypes('/opt/axon/libaxon_pjrt.so'))
